# revision 34
# baseline (speedup 1.0000x reference)
"""Multi-headed attention (B=2, S=2048, D=1024, H=16) on 8 TRN2 NeuronCores.

Sharding: tensor-parallel over heads for the attention body (2 heads/core,
both batches on every core), then AllToAll reshards to seq-slabs: rank r
owns output rows [r*256, (r+1)*256) of BOTH batches. Per core:

  1. K/V/Q projections (bf16 matmuls, fp32 psum):
       qhT/khT [128e, 2048s] (e on partitions), vht [2048t, 128e'].
  2. logits^T = khT-tiles.T @ qhT  (K=64, two heads row-packed via PE row
     tiling: head0 -> psum bank A, head1 -> bank B, concurrent).
  3. P = exp(0.125 * logits^T) on ScalarE (PSUM -> SBUF bf16, FD=1024).
  4. heads^T += vh.T @ P (col-packed over two heads, accumulated over 16
     t-tiles); rowsums += ones.T @ P (replicated over 64 partitions).
  5. rec = 1/rowsum (DVE); heads^T *= rec -> hN bf16.
  6. Two AllToAlls (one per batch) of [8, 128, 256] seq-slabs, fired as
     each batch finishes.  No zero padding: every rank consumes both.
  7. out[b-slab] = gelu_sigmoid(heads_full^T-tiles.T @ Wo + bo) per batch:
     [256, 1024] f32 each; batch-0's projection runs during the second
     AllToAll's wait (with keep-warm dummy matmuls padding the rest).

Batch-1 K/Q projections are dripped into batch-0's attention loop; batch-1
V projection is dripped just-in-time into batch-1's own first attention
quarter, so the Tensor engine stays dense enough to hold the HAM clock at
full rate throughout.
"""

import numpy as np
import ml_dtypes

import concourse.bass as bass
import concourse.mybir as mybir
import concourse.tile as tile
from concourse import bacc
from concourse.bass_utils import run_bass_kernel_spmd

F = mybir.ActivationFunctionType
BF16 = mybir.dt.bfloat16
F8 = mybir.dt.float8e4
F32 = mybir.dt.float32
BF = ml_dtypes.bfloat16
F8NP = ml_dtypes.float8_e4m3

B, S, D, H = 2, 2048, 1024, 16
HD = D // H
KP = 4    # DoubleRow chunk-pairs: contraction 256 per matmul
NCORES = 8
SLAB = S // NCORES          # 256 seq rows per rank per batch
KT = D // 128
TT = S // 128

_CACHE = {}


def _build():
    nc = bacc.Bacc("TRN2", target_bir_lowering=False, debug=False,
                   num_devices=NCORES)
    # q/k activations fp8, DoubleRow-interleaved: [ki, kp, s, ko] where the
    # virtual contraction row is d = kp*256 + ko*128 + ki
    xq = [nc.dram_tensor(f"xq{b}", [128, KP * S * 2], F8, kind="ExternalInput") for b in range(B)]
    xk = [nc.dram_tensor(f"xk{b}", [128, KP * S * 2], F8, kind="ExternalInput") for b in range(B)]
    xv = [nc.dram_tensor(f"xv{b}", [D, S], BF16, kind="ExternalInput") for b in range(B)]
    # wq/wk fp8 DoubleRow-interleaved [ki, kp, ko, m]; wv stays bf16
    wq_d = nc.dram_tensor("wq", [128, KP * 2 * 128], F8, kind="ExternalInput")
    wk_d = nc.dram_tensor("wk", [128, KP * 2 * 128], F8, kind="ExternalInput")
    wv_d = nc.dram_tensor("wv", [128, KT * 128], BF16, kind="ExternalInput")
    bq_d = nc.dram_tensor("bq", [128, 1], F32, kind="ExternalInput")
    bk_d = nc.dram_tensor("bk", [128, 1], F32, kind="ExternalInput")
    bv_d = nc.dram_tensor("bv", [128, 128], BF16, kind="ExternalInput")
    # host pre-arranged to [128, KT*1024]
    wo_d = nc.dram_tensor("wo", [128, KT * D], BF16, kind="ExternalInput")
    bo_d = nc.dram_tensor("bo", [1, D], BF16, kind="ExternalInput")
    onr_d = nc.dram_tensor("onr", [1, 128], BF16, kind="ExternalInput")
    onc_d = nc.dram_tensor("onc", [128, 64], BF16, kind="ExternalInput")
    out_d = nc.dram_tensor("out", [2 * SLAB, D], F32, kind="ExternalOutput")

    xqr = [xq[b][:, :].rearrange("p (kp ko s) -> p kp ko s", kp=KP, ko=2) for b in range(B)]
    xkr = [xk[b][:, :].rearrange("p (kp ko s) -> p kp ko s", kp=KP, ko=2) for b in range(B)]
    xvr = [xv[b][:, :].rearrange("(kt p) s -> kt p s", p=128) for b in range(B)]

    with tile.TileContext(nc) as tc:
        with tc.tile_pool(name="cst", bufs=1) as cst, \
             tc.tile_pool(name="act", bufs=1) as acp, \
             tc.tile_pool(name="str", bufs=4) as stp, \
             tc.tile_pool(name="s2", bufs=3) as s2p, \
             tc.tile_pool(name="ps", bufs=2, space="PSUM") as ps, \
             tc.tile_pool(name="dram", bufs=1, space="DRAM") as dp:

            # K weights + batch-0 K activations first: the first projection
            # matmul can then start as soon as ~1.25 MB has landed.
            wkt = cst.tile([128, KP, 2, 128], F8, tag="wkt")
            nc.sync.dma_start(wkt[:, :, :, :],
                              wk_d[:, :].rearrange("p (kp ko e) -> p kp ko e",
                                                   kp=KP, ko=2))

            qhT = [acp.tile([128, S], BF16, tag=f"qhT{b}", name=f"qhT{b}") for b in range(B)]
            khT = [acp.tile([128, S], BF16, tag=f"khT{b}", name=f"khT{b}") for b in range(B)]
            vht = [acp.tile([128, TT, 128], BF16, tag=f"vht{b}", name=f"vht{b}") for b in range(B)]
            vx = [acp.tile([128, KT, S], BF16, tag="vx", name=f"vx{b}") for b in range(B)]
            hN = [acp.tile([128, S], BF16, tag=f"hN{b}", name=f"hN{b}") for b in range(B)]
            wot = cst.tile([128, KT, D], BF16, tag="wot")

            bqt = cst.tile([128, 1], F32, tag="bqt")
            bkt = cst.tile([128, 1], F32, tag="bkt")
            # bv host-replicated to all 128 partitions: the V bias is added
            # on the DVE during PSUM evacuation instead of a per-t-tile matmul
            bvt = cst.tile([128, 128], BF16, tag="bvt")
            bot = cst.tile([1, D], BF16, tag="bot")
            onr = cst.tile([1, 128], BF16, tag="onr")
            onc = cst.tile([128, 64], BF16, tag="onc")
            wqt = cst.tile([128, KP, 2, 128], F8, tag="wqt")
            wvt = cst.tile([128, KT, 128], BF16, tag="wvt")

            a2a_in = [dp.tile([NCORES, 128, SLAB], BF16, tag=f"a2a_in{b}", name=f"a2a_in{b}")
                      for b in range(B)]
            a2a_out = [dp.tile([NCORES, 128, SLAB], BF16, tag=f"a2a_out{b}", name=f"a2a_out{b}")
                       for b in range(B)]

            # ---------- emission helpers ----------
            def kqproj_steps(b, which, sp, qeng=None):
                """K/Q projection for one 1024-wide s-half, as drip steps.
                qeng picks the engine queue issuing the chunk DMAs, to spread
                descriptor work across queues during the start-up ramp."""
                w_t, b_t, dst, xr, pre = {
                    "k": (wkt, bkt, khT[b], xkr[b], "xk"),
                    "q": (wqt, bqt, qhT[b], xqr[b], "xq"),
                }[which]
                eng = qeng if qeng is not None else nc.sync
                state = {}

                def load():
                    state["xc"] = []
                    for kp in range(KP):
                        xc = stp.tile([128, 2, 1024], F8, tag=pre, bufs=9,
                                      name=f"{pre}{b}{sp}{kp}")
                        eng.dma_start(xc[:, :, :],
                                      xr[:, kp, :, sp * 1024:(sp + 1) * 1024])
                        state["xc"].append(xc)
                yield load

                for half in range(2):
                    def palloc(half=half):
                        state["P"] = ps.tile([128, 512], F32, tag="A",
                                             name=f"{pre}p{b}{sp}{half}")
                        for kp in range(0, 2):
                            nc.tensor.matmul(
                                state["P"][:, :], w_t[:, kp, :, :],
                                state["xc"][kp][:, :, half * 512:(half + 1) * 512],
                                perf_mode=mybir.MatmulPerfMode.DoubleRow,
                                start=(kp == 0), stop=False)
                    yield palloc

                    def pfin(half=half):
                        P = state["P"]
                        for kp in range(2, KP):
                            nc.tensor.matmul(
                                P[:, :], w_t[:, kp, :, :],
                                state["xc"][kp][:, :, half * 512:(half + 1) * 512],
                                perf_mode=mybir.MatmulPerfMode.DoubleRow,
                                start=False, stop=(kp == KP - 1))
                        off = sp * 1024 + half * 512
                        nc.vector.tensor_scalar_add(dst[:, off:off + 512],
                                                    P[:, :], b_t[:, 0:1])
                    yield pfin

            def vload_steps(b):
                for kt in range(KT):
                    def mk(b=b, kt=kt):
                        nc.gpsimd.dma_start(vx[b][:, kt, :], xvr[b][kt, :, :])
                    yield mk

            def vproj_steps(b):
                for tt in range(TT):
                    state = {}

                    def s0(b=b, tt=tt, state=state):
                        state["Vp"] = ps.tile([128, 128], F32, tag="A",
                                              name=f"Vp{b}{tt}")
                        for kt in range(4):
                            nc.tensor.matmul(state["Vp"][:, :],
                                             vx[b][:, kt, tt * 128:(tt + 1) * 128],
                                             wvt[:, kt, :], start=(kt == 0), stop=False)
                    yield s0

                    def s1(b=b, tt=tt, state=state):
                        Vp = state["Vp"]
                        for kt in range(4, KT):
                            nc.tensor.matmul(Vp[:, :],
                                             vx[b][:, kt, tt * 128:(tt + 1) * 128],
                                             wvt[:, kt, :], start=False,
                                             stop=(kt == KT - 1))
                        nc.vector.tensor_add(vht[b][:, tt, :], Vp[:, :],
                                             bvt[:, :])
                    yield s1

            def stage2(b, sc, filler=None):
                s0, s1 = sc * 512, (sc + 1) * 512
                A = ps.tile([128, 512], F32, tag="A", name=f"A{b}{sc}")
                R = ps.tile([128, 512], F32, tag="R", name=f"R{b}{sc}")
                for tt in range(TT):
                    t0, t1 = tt * 128, (tt + 1) * 128
                    L2 = ps.tile([128, 1024], F32, tag="L", name=f"L2{b}{sc}{tt}")
                    nc.tensor.matmul(L2[:, 0:512], khT[b][0:64, t0:t1],
                                     qhT[b][0:64, s0:s1], start=True, stop=True)
                    nc.tensor.matmul(L2[:, 512:1024], khT[b][64:128, t0:t1],
                                     qhT[b][64:128, s0:s1], start=True, stop=True)
                    P = s2p.tile([128, 1024], BF16, tag="P", bufs=4, name=f"P{b}{sc}{tt}")
                    nc.scalar.activation(P[:, :], L2[:, :], F.Exp, scale=0.125)
                    st, sp_ = (tt == 0), (tt == TT - 1)
                    nc.tensor.matmul(A[0:64, :], vht[b][:, tt, 0:64], P[:, 0:512],
                                     start=st, stop=sp_)
                    nc.tensor.matmul(A[64:128, :], vht[b][:, tt, 64:128], P[:, 512:1024],
                                     start=st, stop=sp_)
                    nc.tensor.matmul(R[0:64, :], onc[:, :], P[:, 0:512],
                                     start=st, stop=sp_)
                    nc.tensor.matmul(R[64:128, :], onc[:, :], P[:, 512:1024],
                                     start=st, stop=sp_)
                    if filler is not None:
                        step = next(filler, None)
                        if step is not None:
                            step()
                rec = s2p.tile([128, 512], F32, tag="rec", bufs=2, name=f"rec{b}{sc}")
                nc.vector.reciprocal(rec[:, :], R[:, :])
                nc.vector.tensor_mul(hN[b][:, s0:s1], A[:, :], rec[:, :])
                # two seq-slabs per quarter -> ranks 2*sc and 2*sc+1
                for j in range(2):
                    nc.sync.dma_start(a2a_in[b][2 * sc + j, :, :],
                                      hN[b][:, s0 + j * SLAB:s0 + (j + 1) * SLAB])

            def oproj(b, hf, warm=0):
                """Output projection + gelu + store for this rank's slab of
                batch b.  hf: [128, KT, SLAB] gathered heads.  warm>0 emits
                a block of keep-warm dummy matmuls after each st-group (used
                to span the second AllToAll's wait at full HAM clock)."""
                wsb = s2p.tile([128, 128], BF16, tag="wsb", bufs=2, name=f"wsb{b}")
                for st in range(SLAB // 128):
                    O = ps.tile([128, 1024], F32, tag="L", name=f"O{b}_{st}")
                    for nn in range(2):
                        n0, n1 = nn * 512, (nn + 1) * 512
                        for kt in range(KT):
                            nc.tensor.matmul(O[:, n0:n1],
                                             hf[:, kt, st * 128:(st + 1) * 128],
                                             wot[:, kt, n0:n1],
                                             start=(kt == 0), stop=False)
                        nc.tensor.matmul(O[:, n0:n1], onr[0:1, :], bot[0:1, n0:n1],
                                         start=False, stop=True)
                    OG = s2p.tile([128, 1024], F32, tag="OG", bufs=2,
                                  name=f"OG{b}{st}")
                    nc.scalar.activation(OG[:, :], O[:, :], F.Gelu_apprx_sigmoid)
                    nc.sync.dma_start(
                        out_d[b * SLAB + st * 128:b * SLAB + (st + 1) * 128, :],
                        OG[:, :])
                    if warm:
                        W = ps.tile([128, 512], F32, tag="R", name=f"Wwarm{b}{st}")
                        for i in range(warm):
                            nc.tensor.matmul(W[:, :], wot[:, i % KT, 0:128],
                                             wot[:, i % KT, 0:512],
                                             start=(i == 0), stop=(i == warm - 1))
                        nc.vector.tensor_copy(wsb[:, :], W[:, 0:128])

            # ---------- schedule ----------
            import itertools
            # batch-0 projections: K first (its inputs were queued first),
            # then Q; V loads/projections trail behind on the gpsimd queue.
            # qhT cols 1024:2048 (sp1) are first read by stage2(0, 2), so
            # that projection drips into batch-0 attention instead of the ramp
            pre = itertools.chain(
                kqproj_steps(0, "k", 0), kqproj_steps(0, "k", 1),
                vload_steps(0),
                kqproj_steps(0, "q", 0),
                vproj_steps(0))
            # interleave remaining const loads right after the first k load
            first = next(pre)
            first()
            nc.sync.dma_start(bkt[:, :], bk_d[:, :])
            nc.sync.dma_start(bqt[:, :], bq_d[:, :])
            nc.sync.dma_start(wqt[:, :, :, :],
                              wq_d[:, :].rearrange("p (kp ko e) -> p kp ko e",
                                                   kp=KP, ko=2))
            nc.sync.dma_start(wvt[:, :, :],
                              wv_d[:, :].rearrange("p (kt e) -> p kt e", kt=KT))
            for t, d in ((bvt, bv_d), (bot, bo_d), (onr, onr_d), (onc, onc_d)):
                nc.sync.dma_start(t[:, :], d[:, :])
            for step in pre:
                step()

            # batch-1 projections dripped into batch-0 attention. Everything
            # stage2(1, 0) reads (khT[1], vht[1], qhT[1] first half) must be
            # emitted before it: later-emitted writes would be invisible to
            # the tile dependency tracker (use-before-write).
            fillerA = itertools.chain(kqproj_steps(0, "q", 1),
                                      vload_steps(1),
                                      kqproj_steps(1, "k", 0),
                                      kqproj_steps(1, "k", 1),
                                      kqproj_steps(1, "q", 0),
                                      vproj_steps(1))
            stage2(0, 0, fillerA)
            stage2(0, 1, fillerA)
            stage2(0, 2, fillerA)
            stage2(0, 3, fillerA)
            for step in fillerA:
                step()
            nc.gpsimd.collective_compute(
                "AllToAll", mybir.AluOpType.bypass,
                replica_groups=[list(range(NCORES))],
                ins=[a2a_in[0].opt()], outs=[a2a_out[0].opt()])
            nc.sync.dma_start(wot[:, :, :],
                              wo_d[:, :].rearrange("p (kt n) -> p kt n", kt=KT))
            fillerB = itertools.chain(kqproj_steps(1, "q", 1))
            stage2(1, 0, fillerB)
            stage2(1, 1, fillerB)
            for step in fillerB:
                step()
            stage2(1, 2)
            stage2(1, 3)
            nc.gpsimd.collective_compute(
                "AllToAll", mybir.AluOpType.bypass,
                replica_groups=[list(range(NCORES))],
                ins=[a2a_in[1].opt()], outs=[a2a_out[1].opt()])

            # ---- tail: batch-0 output projection runs during the second
            # AllToAll; dummy warmup matmuls pad the rest of the wait so
            # batch 1's projection starts at full clock.
            hf1 = acp.tile([128, NCORES, SLAB], BF16, tag="hf1")
            for p in range(NCORES):
                nc.sync.dma_start(hf1[:, p, :], a2a_out[0][p, :, :])
            oproj(0, hf1)

            hf2 = acp.tile([128, NCORES, SLAB], BF16, tag="hf2")
            for p in range(NCORES):
                nc.sync.dma_start(hf2[:, p, :], a2a_out[1][p, :, :])
            oproj(1, hf2)

    nc.compile()
    return nc


def _in_maps(q, k, v, Wq, bq, Wk, bk, Wv, bv, Wo, bo):
    def x8(x):
        # [S, D] -> xT [D, S] -> [ki, kp, s, ko] fp8, d = kp*256 + ko*128 + ki
        xT = np.asarray(x).T.reshape(KP, 2, 128, S).transpose(2, 0, 1, 3)
        return np.ascontiguousarray(xT).reshape(128, KP * 2 * S).astype(F8NP)

    xq = [x8(q[b]) for b in range(B)]
    xk = [x8(k[b]) for b in range(B)]
    xv = [np.ascontiguousarray(v[b].T).astype(BF) for b in range(B)]
    # wo: [D, D] -> [128, KT*1024] with row d = kt*128+p  ->  [p, kt, :]
    wo_bf = np.ascontiguousarray(
        np.asarray(Wo).reshape(KT, 128, D).transpose(1, 0, 2).reshape(128, KT * D)
    ).astype(BF)
    bo_r = np.asarray(bo).reshape(1, D).astype(BF)
    onr = np.ones((1, 128), BF)
    onc = np.ones((128, 64), BF)

    def warr(W, hs):
        # [2, D, HD] heads-slice -> [D, 128] -> [128, KT*128] contiguous
        m = np.asarray(W[hs]).transpose(1, 0, 2).reshape(D, 128)
        return np.ascontiguousarray(
            m.reshape(KT, 128, 128).transpose(1, 0, 2).reshape(128, KT * 128)
        ).astype(BF)

    def warr8(W, hs):
        # DoubleRow interleave [ki, kp, ko, m] fp8, d = kp*256 + ko*128 + ki
        m = np.asarray(W[hs]).transpose(1, 0, 2).reshape(D, 128)
        m = m.reshape(KP, 2, 128, 128).transpose(2, 0, 1, 3)
        return np.ascontiguousarray(m).reshape(128, KP * 2 * 128).astype(F8NP)

    in_maps = []
    for c in range(NCORES):
        hs = slice(2 * c, 2 * c + 2)
        im = {
            "wq": warr8(Wq, hs), "wk": warr8(Wk, hs), "wv": warr(Wv, hs),
            "bq": np.asarray(bq[hs]).reshape(128, 1).astype(np.float32),
            "bk": np.asarray(bk[hs]).reshape(128, 1).astype(np.float32),
            "bv": np.tile(np.asarray(bv[hs]).reshape(1, 128), (128, 1)).astype(BF),
            "wo": wo_bf, "bo": bo_r, "onr": onr, "onc": onc,
        }
        for b in range(B):
            im[f"xq{b}"] = xq[b]
            im[f"xk{b}"] = xk[b]
            im[f"xv{b}"] = xv[b]
        in_maps.append(im)
    return in_maps


def kernel(q, k, v, mask, Wq, bq, Wk, bk, Wv, bv, Wo, bo):
    if "nc" not in _CACHE:
        _CACHE["nc"] = _build()
    nc = _CACHE["nc"]
    in_maps = _in_maps(q, k, v, Wq, bq, Wk, bk, Wv, bv, Wo, bo)
    res = run_bass_kernel_spmd(nc, in_maps, core_ids=list(range(NCORES)))
    out = np.empty((B, S, D), np.float32)
    for r in range(NCORES):
        sl = slice(r * SLAB, (r + 1) * SLAB)
        out[0, sl, :] = res.results[r]["out"][:SLAB]
        out[1, sl, :] = res.results[r]["out"][SLAB:]
    return out


# revision 35
# speedup vs baseline: 1.0750x; 1.0750x over previous
"""Multi-headed attention (B=2, S=2048, D=1024, H=16) on 8 TRN2 NeuronCores.

Sharding: tensor-parallel over heads for the attention body (2 heads/core,
both batches on every core), then AllToAll reshards to seq-slabs: rank r
owns output rows [r*256, (r+1)*256) of BOTH batches. Per core:

  1. K/V/Q projections (bf16 matmuls, fp32 psum):
       qhT/khT [128e, 2048s] (e on partitions), vht [2048t, 128e'].
  2. logits^T = khT-tiles.T @ qhT  (K=64, two heads row-packed via PE row
     tiling: head0 -> psum bank A, head1 -> bank B, concurrent).
  3. P = exp(0.125 * logits^T) on ScalarE (PSUM -> SBUF bf16, FD=1024).
  4. heads^T += vh.T @ P (col-packed over two heads, accumulated over 16
     t-tiles); rowsums += ones.T @ P (replicated over 64 partitions).
  5. rec = 1/rowsum (DVE); heads^T *= rec -> hN bf16.
  6. Two AllToAlls (one per batch) of [8, 128, 256] seq-slabs, fired as
     each batch finishes.  No zero padding: every rank consumes both.
  7. out[b-slab] = gelu_sigmoid(heads_full^T-tiles.T @ Wo + bo) per batch:
     [256, 1024] f32 each; batch-0's projection runs during the second
     AllToAll's wait (with keep-warm dummy matmuls padding the rest).

Batch-1 K/Q projections are dripped into batch-0's attention loop; batch-1
V projection is dripped just-in-time into batch-1's own first attention
quarter, so the Tensor engine stays dense enough to hold the HAM clock at
full rate throughout.
"""

import numpy as np
import ml_dtypes

import concourse.bass as bass
import concourse.mybir as mybir
import concourse.tile as tile
from concourse import bacc
from concourse.bass_utils import run_bass_kernel_spmd

F = mybir.ActivationFunctionType
BF16 = mybir.dt.bfloat16
F32 = mybir.dt.float32
BF = ml_dtypes.bfloat16

B, S, D, H = 2, 2048, 1024, 16
HD = D // H
NCORES = 8
SLAB = S // NCORES          # 256 seq rows per rank per batch
KT = D // 128
TT = S // 128

_CACHE = {}


def _build():
    nc = bacc.Bacc("TRN2", target_bir_lowering=False, debug=False,
                   num_devices=NCORES)
    xq = [nc.dram_tensor(f"xq{b}", [D, S], BF16, kind="ExternalInput") for b in range(B)]
    xk = [nc.dram_tensor(f"xk{b}", [D, S], BF16, kind="ExternalInput") for b in range(B)]
    xv = [nc.dram_tensor(f"xv{b}", [D, S], BF16, kind="ExternalInput") for b in range(B)]
    # host pre-arranged to [128, KT*128] so the load is contiguous
    wq_d = nc.dram_tensor("wq", [128, KT * 128], BF16, kind="ExternalInput")
    wk_d = nc.dram_tensor("wk", [128, KT * 128], BF16, kind="ExternalInput")
    wv_d = nc.dram_tensor("wv", [128, KT * 128], BF16, kind="ExternalInput")
    bq_d = nc.dram_tensor("bq", [128, 1], F32, kind="ExternalInput")
    bk_d = nc.dram_tensor("bk", [128, 1], F32, kind="ExternalInput")
    bv_d = nc.dram_tensor("bv", [128, 128], BF16, kind="ExternalInput")
    # host pre-arranged to [128, KT*1024]
    wo_d = nc.dram_tensor("wo", [128, KT * D], BF16, kind="ExternalInput")
    bo_d = nc.dram_tensor("bo", [1, D], BF16, kind="ExternalInput")
    onr_d = nc.dram_tensor("onr", [1, 128], BF16, kind="ExternalInput")
    onc_d = nc.dram_tensor("onc", [128, 64], BF16, kind="ExternalInput")
    out_d = nc.dram_tensor("out", [2 * SLAB, D], F32, kind="ExternalOutput")

    xqr = [xq[b][:, :].rearrange("(kt p) s -> kt p s", p=128) for b in range(B)]
    xkr = [xk[b][:, :].rearrange("(kt p) s -> kt p s", p=128) for b in range(B)]
    xvr = [xv[b][:, :].rearrange("(kt p) s -> kt p s", p=128) for b in range(B)]

    with tile.TileContext(nc) as tc:
        with tc.tile_pool(name="cst", bufs=1) as cst, \
             tc.tile_pool(name="act", bufs=1) as acp, \
             tc.tile_pool(name="str", bufs=4) as stp, \
             tc.tile_pool(name="s2", bufs=3) as s2p, \
             tc.tile_pool(name="ps", bufs=2, space="PSUM") as ps, \
             tc.tile_pool(name="dram", bufs=1, space="DRAM") as dp:

            # K weights + batch-0 K activations first: the first projection
            # matmul can then start as soon as ~1.25 MB has landed.
            wkt = cst.tile([128, KT, 128], BF16, tag="wkt")
            nc.sync.dma_start(wkt[:, :, :],
                              wk_d[:, :].rearrange("p (kt e) -> p kt e", kt=KT))

            qhT = [acp.tile([128, S], BF16, tag=f"qhT{b}", name=f"qhT{b}") for b in range(B)]
            khT = [acp.tile([128, S], BF16, tag=f"khT{b}", name=f"khT{b}") for b in range(B)]
            vht = [acp.tile([128, TT, 128], BF16, tag=f"vht{b}", name=f"vht{b}") for b in range(B)]
            vx = [acp.tile([128, KT, S], BF16, tag="vx", name=f"vx{b}") for b in range(B)]
            hN = [acp.tile([128, S], BF16, tag=f"hN{b}", name=f"hN{b}") for b in range(B)]
            wot = cst.tile([128, KT, D], BF16, tag="wot")

            bqt = cst.tile([128, 1], F32, tag="bqt")
            bkt = cst.tile([128, 1], F32, tag="bkt")
            # bv host-replicated to all 128 partitions: the V bias is added
            # on the DVE during PSUM evacuation instead of a per-t-tile matmul
            bvt = cst.tile([128, 128], BF16, tag="bvt")
            bot = cst.tile([1, D], BF16, tag="bot")
            onr = cst.tile([1, 128], BF16, tag="onr")
            onc = cst.tile([128, 64], BF16, tag="onc")
            wqt = cst.tile([128, KT, 128], BF16, tag="wqt")
            wvt = cst.tile([128, KT, 128], BF16, tag="wvt")

            a2a_in = [dp.tile([NCORES, 128, SLAB], BF16, tag=f"a2a_in{b}", name=f"a2a_in{b}")
                      for b in range(B)]
            a2a_out = [dp.tile([NCORES, 128, SLAB], BF16, tag=f"a2a_out{b}", name=f"a2a_out{b}")
                       for b in range(B)]

            # ---------- emission helpers ----------
            def kqproj_steps(b, which, sp, qeng=None):
                """K/Q projection for one 1024-wide s-half, as drip steps.
                qeng picks the engine queue issuing the chunk DMAs, to spread
                descriptor work across queues during the start-up ramp."""
                w_t, b_t, dst, xr, pre = {
                    "k": (wkt, bkt, khT[b], xkr[b], "xk"),
                    "q": (wqt, bqt, qhT[b], xqr[b], "xq"),
                }[which]
                eng = qeng if qeng is not None else nc.sync
                state = {}

                def load():
                    state["xc"] = []
                    for kt in range(KT):
                        xc = stp.tile([128, 1024], BF16, tag=pre, bufs=9,
                                      name=f"{pre}{b}{sp}{kt}")
                        eng.dma_start(xc[:, :],
                                      xr[kt, :, sp * 1024:(sp + 1) * 1024])
                        state["xc"].append(xc)
                yield load

                for half in range(2):
                    def palloc(half=half):
                        state["P"] = ps.tile([128, 512], F32, tag="A",
                                             name=f"{pre}p{b}{sp}{half}")
                        for kt in range(0, 4):
                            nc.tensor.matmul(state["P"][:, :], w_t[:, kt, :],
                                             state["xc"][kt][:, half * 512:(half + 1) * 512],
                                             start=(kt == 0), stop=False)
                    yield palloc

                    def pfin(half=half):
                        P = state["P"]
                        for kt in range(4, KT):
                            nc.tensor.matmul(P[:, :], w_t[:, kt, :],
                                             state["xc"][kt][:, half * 512:(half + 1) * 512],
                                             start=False, stop=(kt == KT - 1))
                        off = sp * 1024 + half * 512
                        nc.vector.tensor_scalar_add(dst[:, off:off + 512],
                                                    P[:, :], b_t[:, 0:1])
                    yield pfin

            def vload_steps(b):
                for kt in range(KT):
                    def mk(b=b, kt=kt):
                        nc.gpsimd.dma_start(vx[b][:, kt, :], xvr[b][kt, :, :])
                    yield mk

            def vproj_steps(b):
                for tt in range(TT):
                    state = {}

                    def s0(b=b, tt=tt, state=state):
                        state["Vp"] = ps.tile([128, 128], F32, tag="A",
                                              name=f"Vp{b}{tt}")
                        for kt in range(4):
                            nc.tensor.matmul(state["Vp"][:, :],
                                             vx[b][:, kt, tt * 128:(tt + 1) * 128],
                                             wvt[:, kt, :], start=(kt == 0), stop=False)
                    yield s0

                    def s1(b=b, tt=tt, state=state):
                        Vp = state["Vp"]
                        for kt in range(4, KT):
                            nc.tensor.matmul(Vp[:, :],
                                             vx[b][:, kt, tt * 128:(tt + 1) * 128],
                                             wvt[:, kt, :], start=False,
                                             stop=(kt == KT - 1))
                        nc.vector.tensor_add(vht[b][:, tt, :], Vp[:, :],
                                             bvt[:, :])
                    yield s1

            def stage2(b, sc, filler=None):
                s0, s1 = sc * 512, (sc + 1) * 512
                A = ps.tile([128, 512], F32, tag="A", name=f"A{b}{sc}")
                R = ps.tile([128, 512], F32, tag="R", name=f"R{b}{sc}")
                for tt in range(TT):
                    t0, t1 = tt * 128, (tt + 1) * 128
                    L2 = ps.tile([128, 1024], F32, tag="L", name=f"L2{b}{sc}{tt}")
                    nc.tensor.matmul(L2[:, 0:512], khT[b][0:64, t0:t1],
                                     qhT[b][0:64, s0:s1], start=True, stop=True)
                    nc.tensor.matmul(L2[:, 512:1024], khT[b][64:128, t0:t1],
                                     qhT[b][64:128, s0:s1], start=True, stop=True)
                    P = s2p.tile([128, 1024], BF16, tag="P", bufs=4, name=f"P{b}{sc}{tt}")
                    nc.scalar.activation(P[:, :], L2[:, :], F.Exp, scale=0.125)
                    st, sp_ = (tt == 0), (tt == TT - 1)
                    nc.tensor.matmul(A[0:64, :], vht[b][:, tt, 0:64], P[:, 0:512],
                                     start=st, stop=sp_)
                    nc.tensor.matmul(A[64:128, :], vht[b][:, tt, 64:128], P[:, 512:1024],
                                     start=st, stop=sp_)
                    nc.tensor.matmul(R[0:64, :], onc[:, :], P[:, 0:512],
                                     start=st, stop=sp_)
                    nc.tensor.matmul(R[64:128, :], onc[:, :], P[:, 512:1024],
                                     start=st, stop=sp_)
                    if filler is not None:
                        step = next(filler, None)
                        if step is not None:
                            step()
                rec = s2p.tile([128, 512], F32, tag="rec", bufs=2, name=f"rec{b}{sc}")
                nc.vector.reciprocal(rec[:, :], R[:, :])
                nc.vector.tensor_mul(hN[b][:, s0:s1], A[:, :], rec[:, :])
                # two seq-slabs per quarter -> ranks 2*sc and 2*sc+1
                for j in range(2):
                    nc.sync.dma_start(a2a_in[b][2 * sc + j, :, :],
                                      hN[b][:, s0 + j * SLAB:s0 + (j + 1) * SLAB])

            def oproj(b, hf, warm=0):
                """Output projection + gelu + store for this rank's slab of
                batch b.  hf: [128, KT, SLAB] gathered heads.  warm>0 emits
                a block of keep-warm dummy matmuls after each st-group (used
                to span the second AllToAll's wait at full HAM clock)."""
                wsb = s2p.tile([128, 128], BF16, tag="wsb", bufs=2, name=f"wsb{b}")
                for st in range(SLAB // 128):
                    O = ps.tile([128, 1024], F32, tag="L", name=f"O{b}_{st}")
                    for nn in range(2):
                        n0, n1 = nn * 512, (nn + 1) * 512
                        for kt in range(KT):
                            nc.tensor.matmul(O[:, n0:n1],
                                             hf[:, kt, st * 128:(st + 1) * 128],
                                             wot[:, kt, n0:n1],
                                             start=(kt == 0), stop=False)
                        nc.tensor.matmul(O[:, n0:n1], onr[0:1, :], bot[0:1, n0:n1],
                                         start=False, stop=True)
                    OG = s2p.tile([128, 1024], F32, tag="OG", bufs=2,
                                  name=f"OG{b}{st}")
                    nc.scalar.activation(OG[:, :], O[:, :], F.Gelu_apprx_sigmoid)
                    nc.sync.dma_start(
                        out_d[b * SLAB + st * 128:b * SLAB + (st + 1) * 128, :],
                        OG[:, :])
                    if warm:
                        W = ps.tile([128, 512], F32, tag="R", name=f"Wwarm{b}{st}")
                        for i in range(warm):
                            nc.tensor.matmul(W[:, :], wot[:, i % KT, 0:128],
                                             wot[:, i % KT, 0:512],
                                             start=(i == 0), stop=(i == warm - 1))
                        nc.vector.tensor_copy(wsb[:, :], W[:, 0:128])

            # ---------- schedule ----------
            import itertools
            # batch-0 projections: K first (its inputs were queued first),
            # then Q; V loads/projections trail behind on the gpsimd queue.
            # qhT cols 1024:2048 (sp1) are first read by stage2(0, 2), so
            # that projection drips into batch-0 attention instead of the ramp
            pre = itertools.chain(
                kqproj_steps(0, "k", 0), kqproj_steps(0, "k", 1),
                vload_steps(0),
                kqproj_steps(0, "q", 0),
                vproj_steps(0))
            # interleave remaining const loads right after the first k load
            first = next(pre)
            first()
            nc.sync.dma_start(bkt[:, :], bk_d[:, :])
            nc.sync.dma_start(bqt[:, :], bq_d[:, :])
            nc.sync.dma_start(wqt[:, :, :],
                              wq_d[:, :].rearrange("p (kt e) -> p kt e", kt=KT))
            nc.sync.dma_start(wvt[:, :, :],
                              wv_d[:, :].rearrange("p (kt e) -> p kt e", kt=KT))
            for t, d in ((bvt, bv_d), (bot, bo_d), (onr, onr_d), (onc, onc_d)):
                nc.sync.dma_start(t[:, :], d[:, :])
            for step in pre:
                step()

            # batch-1 projections dripped into batch-0 attention. Everything
            # stage2(1, 0) reads (khT[1], vht[1], qhT[1] first half) must be
            # emitted before it: later-emitted writes would be invisible to
            # the tile dependency tracker (use-before-write).
            fillerA = itertools.chain(kqproj_steps(0, "q", 1),
                                      vload_steps(1),
                                      kqproj_steps(1, "k", 0),
                                      kqproj_steps(1, "k", 1),
                                      kqproj_steps(1, "q", 0),
                                      vproj_steps(1))
            stage2(0, 0, fillerA)
            stage2(0, 1, fillerA)
            stage2(0, 2, fillerA)
            stage2(0, 3, fillerA)
            for step in fillerA:
                step()
            nc.gpsimd.collective_compute(
                "AllToAll", mybir.AluOpType.bypass,
                replica_groups=[list(range(NCORES))],
                ins=[a2a_in[0].opt()], outs=[a2a_out[0].opt()])
            nc.sync.dma_start(wot[:, :, :],
                              wo_d[:, :].rearrange("p (kt n) -> p kt n", kt=KT))
            fillerB = itertools.chain(kqproj_steps(1, "q", 1))
            stage2(1, 0, fillerB)
            stage2(1, 1, fillerB)
            for step in fillerB:
                step()
            stage2(1, 2)
            stage2(1, 3)
            nc.gpsimd.collective_compute(
                "AllToAll", mybir.AluOpType.bypass,
                replica_groups=[list(range(NCORES))],
                ins=[a2a_in[1].opt()], outs=[a2a_out[1].opt()])

            # ---- tail: batch-0 output projection runs during the second
            # AllToAll; dummy warmup matmuls pad the rest of the wait so
            # batch 1's projection starts at full clock.
            hf1 = acp.tile([128, NCORES, SLAB], BF16, tag="hf1")
            for p in range(NCORES):
                nc.sync.dma_start(hf1[:, p, :], a2a_out[0][p, :, :])
            oproj(0, hf1)

            hf2 = acp.tile([128, NCORES, SLAB], BF16, tag="hf2")
            for p in range(NCORES):
                nc.sync.dma_start(hf2[:, p, :], a2a_out[1][p, :, :])
            oproj(1, hf2)

    nc.compile()
    return nc


def _in_maps(q, k, v, Wq, bq, Wk, bk, Wv, bv, Wo, bo):
    xq = [np.ascontiguousarray(q[b].T).astype(BF) for b in range(B)]
    xk = [np.ascontiguousarray(k[b].T).astype(BF) for b in range(B)]
    xv = [np.ascontiguousarray(v[b].T).astype(BF) for b in range(B)]
    # wo: [D, D] -> [128, KT*1024] with row d = kt*128+p  ->  [p, kt, :]
    wo_bf = np.ascontiguousarray(
        np.asarray(Wo).reshape(KT, 128, D).transpose(1, 0, 2).reshape(128, KT * D)
    ).astype(BF)
    bo_r = np.asarray(bo).reshape(1, D).astype(BF)
    onr = np.ones((1, 128), BF)
    onc = np.ones((128, 64), BF)

    def warr(W, hs):
        # [2, D, HD] heads-slice -> [D, 128] -> [128, KT*128] contiguous
        m = np.asarray(W[hs]).transpose(1, 0, 2).reshape(D, 128)
        return np.ascontiguousarray(
            m.reshape(KT, 128, 128).transpose(1, 0, 2).reshape(128, KT * 128)
        ).astype(BF)

    in_maps = []
    for c in range(NCORES):
        hs = slice(2 * c, 2 * c + 2)
        im = {
            "wq": warr(Wq, hs), "wk": warr(Wk, hs), "wv": warr(Wv, hs),
            "bq": np.asarray(bq[hs]).reshape(128, 1).astype(np.float32),
            "bk": np.asarray(bk[hs]).reshape(128, 1).astype(np.float32),
            "bv": np.tile(np.asarray(bv[hs]).reshape(1, 128), (128, 1)).astype(BF),
            "wo": wo_bf, "bo": bo_r, "onr": onr, "onc": onc,
        }
        for b in range(B):
            im[f"xq{b}"] = xq[b]
            im[f"xk{b}"] = xk[b]
            im[f"xv{b}"] = xv[b]
        in_maps.append(im)
    return in_maps


def kernel(q, k, v, mask, Wq, bq, Wk, bk, Wv, bv, Wo, bo):
    if "nc" not in _CACHE:
        _CACHE["nc"] = _build()
    nc = _CACHE["nc"]
    in_maps = _in_maps(q, k, v, Wq, bq, Wk, bk, Wv, bv, Wo, bo)
    res = run_bass_kernel_spmd(nc, in_maps, core_ids=list(range(NCORES)))
    out = np.empty((B, S, D), np.float32)
    for r in range(NCORES):
        sl = slice(r * SLAB, (r + 1) * SLAB)
        out[0, sl, :] = res.results[r]["out"][:SLAB]
        out[1, sl, :] = res.results[r]["out"][SLAB:]
    return out
